# revision 1
# baseline (speedup 1.0000x reference)
"""Trainium2 Bass kernel for nn_MiddleOut (gnn_message_passing).

Math (reference):
    out[b,r] = mean_p[ m[b,p] * (my@Wm.T + bias + peer[b,p]@Wp.T + m[b,p]*wm)[r] ]
Collapses to (P = #peers):
    s1[b] = sum_p m[b,p];  s2[b] = sum_p m[b,p]^2
    z[b,l] = sum_p m[b,p] * peer[b,p,l]
    out = (1/P) * [ (s1*my) | z | s2 | s1 ] @ [ Wm.T ; Wp.T ; wm ; bias ]

Sharding: pure data parallel over batch across 8 cores.

On-device strategy per core (Bc=2048 rows, 16 tiles of 128):
  - peer tile host-permuted to [(b4,p)=128 partitions, g=32 groups, l=256]
    (batch b_local = g*4 + b4), cast to bf16 on host (memory-bound problem:
    halves the dominant stream; out rel err ~4e-4), each tile one contiguous
    2MB block so DMA moves 16KB runs per partition.
  - The weighted peer-reduction z runs on the TensorEngine: per group g the
    [128,128] stationary S holds m[g*4+b4, p] at column 4g+b4, rows (b4,p)
    (a zeroed ping-pong tile whose stride-132 diagonal band is rewritten by
    4 DVE copies per tile), so 32 chained matmuls PSUM-accumulate
    psum_z[b_local, l] = sum_p m * peer in natural batch order.
  - s1/s2 from DVE reduce ops, u = s1*my via tensor_scalar.
  - X = [u | z] is PE-transposed in 128-col chunks (fp32-exact), evacuated by
    ACT copies that round to float32r, and fed as stationary into a K=514
    float32r matmul (1 cyc/col vs fp32's 4) against the host-prepacked
    [Wm.T; Wp.T; wm; bias] moving operand, accumulating straight to out.
  - DMA issue is split across the two HWDGE engines (sync: x, scalar:
    meta/out); mt/mb/my are packed into one meta tensor per tile.
"""

import ml_dtypes
import numpy as np

import concourse.bass as bass
import concourse.mybir as mybir
import concourse.tile as tile
from concourse import bacc
from concourse.bass_utils import run_bass_kernel_spmd

F32 = mybir.dt.float32
F32R = mybir.dt.float32r

B, P, L, R = 16384, 32, 256, 256
N_CORES = 8
BC = B // N_CORES          # 2048 batches per core
TILE_B = 128               # batches per SBUF tile
NT = BC // TILE_B          # 16 tiles
G = TILE_B // 4            # 32 groups of 4 batches
NK = 4                     # 128-wide feature chunks of [u|z]


PRECISION = "bf16"   # "f32r": PE-heavy reduced-precision matmuls; "f32": exact


def is_pe_tile(t):
    """Tiles whose peer-reduction runs on the TensorEngine; the rest run a
    DVE multiply-accumulate chain so both engines stay under the DMA floor."""
    if PRECISION in ("f32r", "bf16"):
        return True
    return t % 3 == 0

_cache = {}


def build_bass(nt=NT, num_devices=N_CORES):
    bc = nt * TILE_B
    nc = bacc.Bacc(
        "TRN2", target_bir_lowering=False, debug=False, num_devices=num_devices
    )

    FR = F32R if PRECISION in ("f32r", "bf16") else F32
    BF = mybir.dt.bfloat16
    XD = BF if PRECISION == "bf16" else FR
    x_d = nc.dram_tensor("x", [nt, TILE_B, G, L], XD, kind="ExternalInput")
    # meta packs [mt | mb | my] per tile: one DMA instead of three
    meta_d = nc.dram_tensor(
        "meta", [nt, TILE_B, G + P + L], F32, kind="ExternalInput"
    )
    w_d = nc.dram_tensor("wext", [5, TILE_B, R], FR, kind="ExternalInput")
    id_d = nc.dram_tensor("ident", [TILE_B, TILE_B], F32, kind="ExternalInput")
    out_d = nc.dram_tensor("out", [bc, R], F32, kind="ExternalOutput")

    with TileCtx(nc) as (tc, ctx):
        singles = ctx.enter_context(tc.tile_pool(name="singles", bufs=1))
        xp = ctx.enter_context(tc.tile_pool(name="xp", bufs=6))
        small = ctx.enter_context(tc.tile_pool(name="small", bufs=6))
        xtp = ctx.enter_context(tc.tile_pool(name="xtp", bufs=4))
        psz = ctx.enter_context(tc.tile_pool(name="psz", bufs=3, space="PSUM"))
        pst = ctx.enter_context(tc.tile_pool(name="pst", bufs=2, space="PSUM"))
        pso = ctx.enter_context(tc.tile_pool(name="pso", bufs=3, space="PSUM"))

        w_sb = singles.tile([TILE_B, 5, R], FR)
        nc.sync.dma_start(out=w_sb, in_=w_d.rearrange("k p r -> p k r"))
        ident = singles.tile([TILE_B, TILE_B], F32)
        nc.sync.dma_start(out=ident, in_=id_d[:, :])

        # Ping-pong block-diagonal stationaries for the weighted peer-reduce.
        # s[:, g, :] is [128, 128]: column 4g+b4 holds m[g*4+b4, p] at rows
        # (b4, p); the zeros are written once, the diagonal band is rewritten
        # every tile. f32r matmuls need the full M=128 stationary.
        s_tiles = []
        for i in range(3):
            s_i = singles.tile([TILE_B, G, TILE_B], XD, tag=f"s{i}")
            if PRECISION == "bf16":
                nc.vector.memset(s_i, 0.0)
            else:
                nc.vector.memset(s_i.bitcast(F32), 0.0)
            s_tiles.append(s_i)

        for t in range(nt):
            # ---- loads ----
            if is_pe_tile(t):
                x_t = xp.tile([TILE_B, G, L], XD, tag="x_t")
                nc.sync.dma_start(out=x_t[:, 0:G // 2, :], in_=x_d[t, :, 0:G // 2, :])
                nc.sync.dma_start(out=x_t[:, G // 2:, :], in_=x_d[t, :, G // 2:, :])
            elif PRECISION == "bf16":
                x_t = xp.tile([TILE_B, G, L], BF, tag="x_t")
                nc.sync.dma_start(out=x_t, in_=x_d[t])
            else:
                x_t = xp.tile([TILE_B, G, L], F32, tag="x_t")
                nc.sync.dma_start(out=x_t, in_=x_d[t].bitcast(F32))
            meta = small.tile([TILE_B, G + P + L], F32, tag="meta")
            nc.scalar.dma_start(out=meta, in_=meta_d[t])
            m_t = meta[:, 0:G]
            m_b = meta[:, G:G + P]
            my_t = meta[:, G + P:]

            psum_z = None
            if is_pe_tile(t):
                # ---- fill the diagonal band of S with this tile's metrics ----
                s_all = s_tiles[t % 3]
                for b4 in range(4):
                    view = s_all[b4 * P:(b4 + 1) * P, :, :]
                    out_ap = bass.AP(
                        tensor=view.tensor, offset=view.offset + b4,
                        ap=[view.ap[0], [132, G]],
                    )
                    nc.vector.tensor_copy(
                        out=out_ap, in_=m_t[b4 * P:(b4 + 1) * P, :],
                    )

                # ---- z via PE: psum_z[b_local, l] = sum_p m * peer ----
                # one 32-matmul f32r accumulation chain, M=128
                psum_z = psz.tile([TILE_B, L], F32, tag="psum_z")
                for g in range(G):
                    nc.tensor.matmul(
                        out=psum_z,
                        lhsT=s_all[:, g, :],
                        rhs=x_t[:, g, :],
                        start=(g == 0),
                        stop=(g == G - 1),
                    )

            # ---- s1, s2, u ----
            s12 = small.tile([TILE_B, 2], F32, tag="s12")  # [s2 | s1]
            m2 = small.tile([TILE_B, P], F32, tag="m2")
            nc.vector.tensor_mul(m2, m_b, m_b)
            nc.vector.tensor_reduce(
                out=s12[:, 0:1], in_=m2, axis=mybir.AxisListType.X,
                op=mybir.AluOpType.add,
            )
            nc.vector.tensor_reduce(
                out=s12[:, 1:2], in_=m_b, axis=mybir.AxisListType.X,
                op=mybir.AluOpType.add,
            )

            x_sb = small.tile([TILE_B, 2 * L], F32, tag="x_sb")  # [u | z]
            nc.vector.tensor_scalar_mul(
                out=x_sb[:, 0:L], in0=my_t, scalar1=s12[:, 1:2]
            )
            if is_pe_tile(t):
                nc.scalar.copy(out=x_sb[:, L:2 * L], in_=psum_z)
            else:
                # ---- z via DVE: two interleaved MAC chains (plain [b,p,l]) ----
                acc0 = small.tile([TILE_B, L], F32, tag="acc0")
                acc1 = small.tile([TILE_B, L], F32, tag="acc1")
                nc.vector.tensor_scalar_mul(
                    out=acc0, in0=x_t[:, 0, :], scalar1=m_b[:, 0:1]
                )
                nc.vector.tensor_scalar_mul(
                    out=acc1, in0=x_t[:, 1, :], scalar1=m_b[:, 1:2]
                )
                for p in range(2, P):
                    acc = acc0 if p % 2 == 0 else acc1
                    nc.vector.scalar_tensor_tensor(
                        out=acc, in0=x_t[:, p, :], scalar=m_b[:, p:p + 1],
                        in1=acc, op0=mybir.AluOpType.mult,
                        op1=mybir.AluOpType.add,
                    )
                nc.vector.tensor_add(x_sb[:, L:2 * L], acc0, acc1)

            # ---- transpose X chunks, matmul against packed weights ----
            xts = []
            for k in range(NK):
                pt = pst.tile([TILE_B, TILE_B], F32, tag="pt")
                nc.tensor.transpose(
                    out=pt, in_=x_sb[:, k * TILE_B:(k + 1) * TILE_B],
                    identity=ident,
                )
                xt = xtp.tile([TILE_B, TILE_B], FR, tag=f"xt{k}")
                nc.scalar.copy(out=xt, in_=pt)
                xts.append(xt)
            pt4 = pst.tile([TILE_B, TILE_B], F32, tag="pt")
            nc.tensor.transpose(out=pt4[0:2, :], in_=s12, identity=ident)
            xt4 = xtp.tile([TILE_B, TILE_B], FR, tag="xt4")
            nc.scalar.copy(out=xt4[0:2, :], in_=pt4[0:2, :])

            psum_o = pso.tile([TILE_B, R], F32, tag="psum_o")
            for k in range(NK):
                nc.tensor.matmul(
                    out=psum_o, lhsT=xts[k], rhs=w_sb[:, k, :],
                    start=(k == 0), stop=False,
                )
            nc.tensor.matmul(
                out=psum_o, lhsT=xt4[0:2, :], rhs=w_sb[0:2, 4, :],
                start=False, stop=True,
            )

            out_sb = small.tile([TILE_B, R], F32, tag="out_sb")
            nc.scalar.activation(
                out=out_sb, in_=psum_o,
                func=mybir.ActivationFunctionType.Copy, scale=1.0 / P,
            )
            nc.scalar.dma_start(
                out=out_d[t * TILE_B:(t + 1) * TILE_B, :], in_=out_sb
            )

    nc.compile()
    return nc


class TileCtx:
    """with TileCtx(nc) as (tc, ctx): — TileContext plus an ExitStack."""

    def __init__(self, nc):
        from contextlib import ExitStack
        self.tc = tile.TileContext(nc)
        self.ctx = ExitStack()

    def __enter__(self):
        return self.tc.__enter__(), self.ctx.__enter__()

    def __exit__(self, *a):
        self.ctx.__exit__(*a)
        return self.tc.__exit__(*a)


def prep_inputs(my_latent, peer_latents, peer_metrics, W, b):
    """Host-side shard + layout prep (no arithmetic beyond weight packing)."""
    wext = np.zeros((5, TILE_B, R), dtype=np.float32)
    wt = np.ascontiguousarray(W.T)                       # [513, 256]
    wext.reshape(5 * TILE_B, R)[0:2 * L] = wt[0:2 * L]
    wext[4, 0] = W[:, 2 * L]                             # wm
    wext[4, 1] = b                                       # bias
    ident = np.eye(TILE_B, dtype=np.float32)

    in_maps = []
    for c in range(N_CORES):
        sl = slice(c * BC, (c + 1) * BC)
        # Each tile is one contiguous 4MB block (32KB per partition row).
        # PE tiles: [(b4,p)=128 partitions, g, l]; DVE tiles: plain [b, p, l].
        xdt = ml_dtypes.bfloat16 if PRECISION == "bf16" else np.float32
        plain = peer_latents[sl].reshape(NT, TILE_B, P, L)
        xc = np.empty((NT, TILE_B, G, L), dtype=xdt)
        for t in range(NT):
            if is_pe_tile(t):
                xc[t] = plain[t].reshape(G, 4, P, L).transpose(
                    1, 2, 0, 3).reshape(TILE_B, G, L)
            else:
                xc[t] = plain[t]
        mc = peer_metrics[sl]
        meta = np.empty((NT, TILE_B, G + P + L), dtype=np.float32)
        meta[:, :, 0:G] = mc.reshape(NT, G, 4, P).transpose(
            0, 2, 3, 1).reshape(NT, TILE_B, G)
        meta[:, :, G:G + P] = mc.reshape(NT, TILE_B, P)
        meta[:, :, G + P:] = my_latent[sl].reshape(NT, TILE_B, L)
        in_maps.append({
            "x": xc,
            "meta": meta,
            "wext": wext,
            "ident": ident,
        })
    return in_maps


def run(my_latent, peer_latents, peer_metrics, W, b, trace=False, **kw):
    if "nc" not in _cache:
        _cache["nc"] = build_bass()
    nc = _cache["nc"]
    in_maps = prep_inputs(
        np.asarray(my_latent, dtype=np.float32),
        np.asarray(peer_latents, dtype=np.float32),
        np.asarray(peer_metrics, dtype=np.float32),
        np.asarray(W, dtype=np.float32),
        np.asarray(b, dtype=np.float32),
    )
    res = run_bass_kernel_spmd(
        nc, in_maps, core_ids=list(range(N_CORES)), trace=trace, **kw
    )
    out = np.concatenate([r["out"] for r in res.results], axis=0)
    return out, res


def kernel(my_latent, peer_latents, peer_metrics, W, b):
    out, _ = run(my_latent, peer_latents, peer_metrics, W, b)
    return out



# revision 7
# speedup vs baseline: 1.5378x; 1.5378x over previous
"""Trainium2 Bass kernel for nn_MiddleOut (gnn_message_passing).

Math (reference), with P = #peers and W = [Wm | Wp | wm] along the in dim:
    out = (1/P) * [ s1*(my@Wm.T + bias) + z@Wp.T + s2*wm ]
    s1[b] = sum_p m[b,p];  s2[b] = sum_p m[b,p]^2
    z[b,l] = sum_p m[b,p] * peer[b,p,l]

Sharding: pure data parallel over batch across 8 cores (2048 rows/core,
16 tiles of 128).

On-device strategy per tile (the key changes vs the 172us baseline):
  - peers are cast to fp8 e3m4 on host (4 mantissa bits; |peer|max ~5.4 well
    under the 15.5 format max) halving the dominant HBM stream vs bf16;
    measured end-to-end absmax-rel ~4.7e-3 vs the 2e-2 budget.
  - the weighted peer-reduction z runs as FOUR CONCURRENT column-tiled
    matmul chains (tile_position col groups j=0..3, 32 output partitions
    each): stationary = a [128,32] metric band slab, moving = the [128,256]
    peer slab. Column tiling streams 4 moving operands through the PE at
    once, cutting the z wall-time ~4x vs one 128-wide chain.
    Batch mapping: b_loc = 32*j + 8*b4 + k, slab s = 4*k + j, band column
    c = k + 8*b4 holds m[b_loc, p] at rows (b4,p).
  - projection avoids transposing [u|z]: my is host-transposed (myT) so
    A = my@Wm'/P comes from 2 direct chains; only z (2 chunks, bf16) and
    s12 (tiny) are PE-transposed, then 3 chains give
    psum_o = z@Wp' + s1*bias' + s2*wm' (all 1/P pre-folded on host).
  - final combine on DVE: out = A_sb * s1 + psum_o via scalar_tensor_tensor
    (s1, s2 are exact f32 DVE reductions of the metrics).
"""

import ml_dtypes
import numpy as np

import concourse.bass as bass
import concourse.mybir as mybir
import concourse.tile as tile
from concourse import bacc
from concourse.bass_utils import run_bass_kernel_spmd

F32 = mybir.dt.float32
BF = mybir.dt.bfloat16
E3 = mybir.dt.float8e3

B, P, L, R = 16384, 32, 256, 256
N_CORES = 8
BC = B // N_CORES          # 2048 batches per core
TILE_B = 128               # batches per SBUF tile
NT = BC // TILE_B          # 16 tiles
NJ = 4                     # column-tile groups (32 out partitions each)
NK = 8                     # chained matmuls per column group
NS = NJ * NK               # 32 peer slabs per tile

_cache = {}


def build_bass(nt=NT, num_devices=N_CORES):
    bc = nt * TILE_B
    nc = bacc.Bacc(
        "TRN2", target_bir_lowering=False, debug=False, num_devices=num_devices
    )

    x_d = nc.dram_tensor("x", [nt, TILE_B, NS, L], E3, kind="ExternalInput")
    # meta packs [m_t | mb] per tile: m_t feeds the band writes, mb the s1/s2
    meta_d = nc.dram_tensor("meta", [nt, TILE_B, 2 * P], F32, kind="ExternalInput")
    myt_d = nc.dram_tensor("myt", [nt, TILE_B, 2, TILE_B], BF, kind="ExternalInput")
    # wbf packs [WmT' | WpT' | identity | w45'] (bf16); identf is f32 identity
    wbf_d = nc.dram_tensor("wbf", [TILE_B, 4 * L + TILE_B + R], BF, kind="ExternalInput")
    identf_d = nc.dram_tensor("identf", [TILE_B, TILE_B], F32, kind="ExternalInput")
    out_d = nc.dram_tensor("out", [bc, R], F32, kind="ExternalOutput")

    with TileCtx(nc) as (tc, ctx):
        singles = ctx.enter_context(tc.tile_pool(name="singles", bufs=1))
        xp = ctx.enter_context(tc.tile_pool(name="xp", bufs=5))
        small = ctx.enter_context(tc.tile_pool(name="small", bufs=4))
        xtp = ctx.enter_context(tc.tile_pool(name="xtp", bufs=3))
        psz = ctx.enter_context(tc.tile_pool(name="psz", bufs=2, space="PSUM"))
        pst = ctx.enter_context(tc.tile_pool(name="pst", bufs=2, space="PSUM"))
        pso = ctx.enter_context(tc.tile_pool(name="pso", bufs=2, space="PSUM"))
        psa = ctx.enter_context(tc.tile_pool(name="psa", bufs=1, space="PSUM"))

        w_sb = singles.tile([TILE_B, 4 * L + TILE_B + R], BF)
        nc.sync.dma_start(out=w_sb, in_=wbf_d[:, :])
        identf = singles.tile([TILE_B, TILE_B], F32)
        nc.sync.dma_start(out=identf, in_=identf_d[:, :])
        wm_sb = w_sb[:, 0:2 * L]                     # [128, 2*256] WmT'/P
        wp_sb = w_sb[:, 2 * L:4 * L]                 # [128, 2*256] WpT'/P
        identb = w_sb[:, 4 * L:4 * L + TILE_B]       # [128, 128] bf16 identity
        w45 = w_sb[:, 4 * L + TILE_B:]               # rows 0:2 = [bias'; wm']/P

        # Ping-pong block-diagonal stationaries for the weighted peer-reduce.
        # Slab s = 4k+j is [128, 32]: column k+8*b4 holds m[32j+8b4+k, p] at
        # rows (b4,p); zeros written once, the band rewritten every tile.
        s_tiles = []
        for i in range(3):
            s_i = singles.tile([TILE_B, NS, P], E3, tag=f"s{i}")
            nc.vector.memset(s_i.bitcast(F32), 0.0)
            s_tiles.append(s_i)

        for t in range(nt):
            # ---- loads ----
            x_t = xp.tile([TILE_B, NS, L], E3, tag="x_t")
            nc.sync.dma_start(out=x_t[:, 0:NS // 2, :], in_=x_d[t, :, 0:NS // 2, :])
            nc.sync.dma_start(out=x_t[:, NS // 2:, :], in_=x_d[t, :, NS // 2:, :])
            meta = small.tile([TILE_B, 2 * P], F32, tag="meta")
            nc.scalar.dma_start(out=meta, in_=meta_d[t])
            myt = small.tile([TILE_B, 2, TILE_B], BF, tag="myt")
            nc.scalar.dma_start(out=myt, in_=myt_d[t])

            # ---- fill the diagonal band of the slabs with this tile's metrics
            # element (32*b4+p, slab 4k+j, col k+8*b4): free off = 129k+32j+8b4
            s_all = s_tiles[t % 3]
            for b4 in range(4):
                view = s_all[b4 * P:(b4 + 1) * P, :, :]
                out_ap = bass.AP(
                    tensor=view.tensor, offset=view.offset + 8 * b4,
                    ap=[view.ap[0], [32, NJ], [129, NK]],
                )
                mtv = meta[b4 * P:(b4 + 1) * P, 0:P]
                in_ap = bass.AP(
                    tensor=mtv.tensor, offset=mtv.offset,
                    ap=[mtv.ap[0], [1, NJ], [4, NK]],
                )
                nc.vector.tensor_copy(out=out_ap, in_=in_ap)

            # ---- s1, s2 (exact, f32) ----
            mb = meta[:, P:2 * P]
            m2 = small.tile([TILE_B, P], F32, tag="m2")
            nc.vector.tensor_mul(m2, mb, mb)
            s12 = small.tile([TILE_B, 2], F32, tag="s12")  # [s1 | s2]
            nc.vector.tensor_reduce(
                out=s12[:, 0:1], in_=mb, axis=mybir.AxisListType.X,
                op=mybir.AluOpType.add,
            )
            nc.vector.tensor_reduce(
                out=s12[:, 1:2], in_=m2, axis=mybir.AxisListType.X,
                op=mybir.AluOpType.add,
            )

            # ---- z via 4 concurrent column-tiled accumulation chains ----
            psum_z = psz.tile([TILE_B, L], F32, tag="psum_z")
            for k in range(NK):
                for j in range(NJ):
                    s = 4 * k + j
                    nc.tensor.matmul(
                        out=psum_z[32 * j:32 * (j + 1), :],
                        lhsT=s_all[:, s, :],
                        rhs=x_t[:, s, :],
                        start=(k == 0),
                        stop=(k == NK - 1),
                        tile_position=(0, 32 * j),
                        skip_group_check=True,
                    )

            # ---- A = my @ WmT'  (no transpose needed: myT from host) ----
            psum_a = psa.tile([TILE_B, R], F32, tag="psum_a")
            nc.tensor.matmul(
                out=psum_a, lhsT=myt[:, 0, :], rhs=wm_sb[:, 0:R],
                start=True, stop=False,
            )
            nc.tensor.matmul(
                out=psum_a, lhsT=myt[:, 1, :], rhs=wm_sb[:, R:2 * R],
                start=False, stop=True,
            )

            # ---- transpose z (2 bf16 chunks) and s12 (f32) ----
            z_sb = small.tile([TILE_B, L], BF, tag="z_sb")
            nc.scalar.copy(out=z_sb, in_=psum_z)
            zts = []
            for c in range(2):
                ptz = pst.tile([TILE_B, TILE_B], BF, tag="ptz")
                nc.tensor.transpose(
                    out=ptz, in_=z_sb[:, c * TILE_B:(c + 1) * TILE_B],
                    identity=identb,
                )
                zt = xtp.tile([TILE_B, TILE_B], BF, tag=f"zt{c}")
                nc.scalar.copy(out=zt, in_=ptz)
                zts.append(zt)
            pts = pst.tile([TILE_B, TILE_B], F32, tag="pts", bufs=1)
            nc.tensor.transpose(out=pts[0:2, :], in_=s12, identity=identf)
            s12t = xtp.tile([2, TILE_B], BF, tag="s12t")
            nc.scalar.copy(out=s12t, in_=pts[0:2, :])

            # ---- psum_o = z@WpT' + s1*bias' + s2*wm' ----
            psum_o = pso.tile([TILE_B, R], F32, tag="psum_o")
            nc.tensor.matmul(
                out=psum_o, lhsT=zts[0], rhs=wp_sb[:, 0:R],
                start=True, stop=False,
            )
            nc.tensor.matmul(
                out=psum_o, lhsT=zts[1], rhs=wp_sb[:, R:2 * R],
                start=False, stop=False,
            )
            nc.tensor.matmul(
                out=psum_o, lhsT=s12t, rhs=w45[0:2, :],
                start=False, stop=True,
            )

            # ---- out = A_sb * s1 + psum_o  (DVE), then store ----
            a_sb = small.tile([TILE_B, R], BF, tag="a_sb")
            nc.scalar.copy(out=a_sb, in_=psum_a)
            out_sb = small.tile([TILE_B, R], F32, tag="out_sb")
            nc.vector.scalar_tensor_tensor(
                out=out_sb, in0=a_sb, scalar=s12[:, 0:1], in1=psum_o,
                op0=mybir.AluOpType.mult, op1=mybir.AluOpType.add,
            )
            nc.scalar.dma_start(
                out=out_d[t * TILE_B:(t + 1) * TILE_B, :], in_=out_sb
            )

    nc.compile()
    return nc


class TileCtx:
    """with TileCtx(nc) as (tc, ctx): — TileContext plus an ExitStack."""

    def __init__(self, nc):
        from contextlib import ExitStack
        self.tc = tile.TileContext(nc)
        self.ctx = ExitStack()

    def __enter__(self):
        return self.tc.__enter__(), self.ctx.__enter__()

    def __exit__(self, *a):
        self.ctx.__exit__(*a)
        return self.tc.__exit__(*a)


def prep_inputs(my_latent, peer_latents, peer_metrics, W, b, nt=NT, n_cores=N_CORES):
    """Host-side shard + layout prep (dtype casts and permutes; 1/P folded
    into the weight pack)."""
    E3np = ml_dtypes.float8_e3m4
    bc = nt * TILE_B

    wbf = np.zeros((TILE_B, 4 * L + TILE_B + R), dtype=np.float32)
    wt = np.ascontiguousarray(W.T) / P                   # [513, 256] pre-scaled
    # WmT' chunks: [:, c*256:(c+1)*256][lp, r] = W[r, c*128+lp]/P
    wbf[:, 0:R] = wt[0:TILE_B]
    wbf[:, R:2 * R] = wt[TILE_B:2 * TILE_B]
    wbf[:, 2 * L:2 * L + R] = wt[L:L + TILE_B]
    wbf[:, 2 * L + R:4 * L] = wt[L + TILE_B:L + 2 * TILE_B]
    wbf[:, 4 * L:4 * L + TILE_B] = np.eye(TILE_B, dtype=np.float32)
    wbf[0, 4 * L + TILE_B:] = b / P                      # bias'
    wbf[1, 4 * L + TILE_B:] = wt[2 * L]                  # wm'
    wbf = wbf.astype(ml_dtypes.bfloat16)
    identf = np.eye(TILE_B, dtype=np.float32)

    # batch scramble within a tile: b_loc = 32j + 8*b4 + k
    b4r = np.arange(4)[:, None, None]
    kr = np.arange(NK)[None, :, None]
    jr = np.arange(NJ)[None, None, :]
    bl_map = 32 * jr + 8 * b4r + kr                      # [4, 8, 4] (b4, k, j)

    in_maps = []
    for c in range(n_cores):
        sl = slice(c * bc, (c + 1) * bc)
        peer_c = peer_latents[sl].reshape(nt, TILE_B, P, L)
        m_c = peer_metrics[sl].reshape(nt, TILE_B, P)
        my_c = my_latent[sl].reshape(nt, TILE_B, L)

        # x[t, 32*b4+p, 4k+j, l] = peer[bl_map, p, l]
        xc = peer_c[:, bl_map, :, :]                     # [nt, 4, 8, 4, P, L]
        xc = np.ascontiguousarray(
            xc.transpose(0, 1, 4, 2, 3, 5)               # t, b4, p, k, j, l
        ).reshape(nt, TILE_B, NS, L).astype(E3np)

        # meta: [m_t | mb]; m_t[t, 32*b4+p, 4k+j] = m[bl_map, p]
        meta = np.empty((nt, TILE_B, 2 * P), dtype=np.float32)
        mt = m_c[:, bl_map, :]                           # [nt, 4, 8, 4, P]
        meta[:, :, 0:P] = mt.transpose(0, 1, 4, 2, 3).reshape(nt, TILE_B, P)
        meta[:, :, P:2 * P] = m_c
        # myT[t, lp, c2, b] = my[b, c2*128+lp]
        myt = np.ascontiguousarray(
            my_c.reshape(nt, TILE_B, 2, TILE_B).transpose(0, 3, 2, 1)
        ).astype(ml_dtypes.bfloat16)

        in_maps.append({
            "x": xc,
            "meta": meta,
            "myt": myt,
            "wbf": wbf,
            "identf": identf,
        })
    return in_maps


def run(my_latent, peer_latents, peer_metrics, W, b, trace=False, **kw):
    if "nc" not in _cache:
        _cache["nc"] = build_bass()
    nc = _cache["nc"]
    in_maps = prep_inputs(
        np.asarray(my_latent, dtype=np.float32),
        np.asarray(peer_latents, dtype=np.float32),
        np.asarray(peer_metrics, dtype=np.float32),
        np.asarray(W, dtype=np.float32),
        np.asarray(b, dtype=np.float32),
    )
    res = run_bass_kernel_spmd(
        nc, in_maps, core_ids=list(range(N_CORES)), trace=trace, **kw
    )
    out = np.concatenate([r["out"] for r in res.results], axis=0)
    return out, res


def kernel(my_latent, peer_latents, peer_metrics, W, b):
    out, _ = run(my_latent, peer_latents, peer_metrics, W, b)
    return out


# revision 8
# speedup vs baseline: 1.8451x; 1.1999x over previous
"""Trainium2 Bass kernel for nn_MiddleOut (gnn_message_passing).

Math (reference), with P = #peers and W = [Wm | Wp | wm] along the in dim:
    out = (1/P) * [ s1*(my@Wm.T + bias) + z@Wp.T + s2*wm ]
    s1[b] = sum_p m[b,p];  s2[b] = sum_p m[b,p]^2
    z[b,l] = sum_p m[b,p] * peer[b,p,l]

Sharding: pure data parallel over batch across 8 cores (2048 rows/core,
16 tiles of 128).

On-device strategy per tile:
  - peers are cast to fp8 e3m4 on host (4 mantissa bits; |peer|max ~5.4 well
    under the 15.5 format max), halving the dominant HBM stream vs bf16;
    measured end-to-end absmax-rel ~4.3e-3 vs the 2e-2 budget.
  - the weighted peer-reduction z runs as FOUR CONCURRENT column-tiled
    matmul chains (tile_position col groups j=0..3, 32 output partitions
    each): stationary = a [128,32] metric band slab, moving = the [128,256]
    peer slab. Column tiling streams 4 moving operands through the PE at
    once, cutting the z wall-time ~4x vs one 128-wide chain.
    Batch mapping: b_loc = 32*j + 8*b4 + k, slab s = 4*k + j, band column
    c = k + 8*b4 holds m[b_loc, p] at rows (b4,p).
  - projection avoids transposing [u|z]: my is host-transposed (myT) so
    A = my@Wm'/P comes from 2 direct chains; only z (2 chunks, bf16) and
    s12 (tiny) are PE-transposed, then 3 chains give
    psum_o = z@Wp' + s1*bias' + s2*wm' (all 1/P pre-folded on host).
  - final combine on DVE: out = A_sb * s1 + psum_o via scalar_tensor_tensor
    (s1, s2 are f32 DVE reductions of the bf16 metrics).
  - DMA issue cost (~0.6us of sequencer time per dma_start) is kept off the
    busy engines: x is one 1MB issue on sync; the merged meta|myT load and
    the out store issue from the otherwise-idle GpSimd (SWDGE).
"""

import ml_dtypes
import numpy as np

import concourse.bass as bass
import concourse.mybir as mybir
import concourse.tile as tile
from concourse import bacc
from concourse.bass_utils import run_bass_kernel_spmd

F32 = mybir.dt.float32
BF = mybir.dt.bfloat16
E3 = mybir.dt.float8e3

B, P, L, R = 16384, 32, 256, 256
N_CORES = 8
BC = B // N_CORES          # 2048 batches per core
TILE_B = 128               # batches per SBUF tile
NT = BC // TILE_B          # 16 tiles
NJ = 4                     # column-tile groups (32 out partitions each)
NK = 8                     # chained matmuls per column group
NS = NJ * NK               # 32 peer slabs per tile
MW = 2 * P + 2 * TILE_B    # meta row: [m_t | mb | myT chunk0 | myT chunk1]

_cache = {}


def build_bass(nt=NT, num_devices=N_CORES):
    bc = nt * TILE_B
    nc = bacc.Bacc(
        "TRN2", target_bir_lowering=False, debug=False, num_devices=num_devices
    )

    x_d = nc.dram_tensor("x", [nt, TILE_B, NS, L], E3, kind="ExternalInput")
    meta_d = nc.dram_tensor("meta", [nt, TILE_B, MW], BF, kind="ExternalInput")
    # wbf packs [WmT' | WpT' | identity | w45'] (bf16); identf is f32 identity
    wbf_d = nc.dram_tensor("wbf", [TILE_B, 4 * L + TILE_B + R], BF, kind="ExternalInput")
    identf_d = nc.dram_tensor("identf", [TILE_B, TILE_B], F32, kind="ExternalInput")
    out_d = nc.dram_tensor("out", [bc, R], F32, kind="ExternalOutput")

    with TileCtx(nc) as (tc, ctx):
        singles = ctx.enter_context(tc.tile_pool(name="singles", bufs=1))
        xp = ctx.enter_context(tc.tile_pool(name="xp", bufs=5))
        small = ctx.enter_context(tc.tile_pool(name="small", bufs=4))
        xtp = ctx.enter_context(tc.tile_pool(name="xtp", bufs=3))
        psz = ctx.enter_context(tc.tile_pool(name="psz", bufs=2, space="PSUM"))
        pst = ctx.enter_context(tc.tile_pool(name="pst", bufs=2, space="PSUM"))
        pso = ctx.enter_context(tc.tile_pool(name="pso", bufs=2, space="PSUM"))
        psa = ctx.enter_context(tc.tile_pool(name="psa", bufs=1, space="PSUM"))

        w_sb = singles.tile([TILE_B, 4 * L + TILE_B + R], BF)
        nc.gpsimd.dma_start(out=w_sb, in_=wbf_d[:, :])
        identf = singles.tile([TILE_B, TILE_B], F32)
        nc.gpsimd.dma_start(out=identf, in_=identf_d[:, :])
        wm_sb = w_sb[:, 0:2 * L]                     # [128, 2*256] WmT'/P
        wp_sb = w_sb[:, 2 * L:4 * L]                 # [128, 2*256] WpT'/P
        identb = w_sb[:, 4 * L:4 * L + TILE_B]       # [128, 128] bf16 identity
        w45 = w_sb[:, 4 * L + TILE_B:]               # rows 0:2 = [bias'; wm']/P

        # Ping-pong block-diagonal stationaries for the weighted peer-reduce.
        # Slab s = 4k+j is [128, 32]: column k+8*b4 holds m[32j+8b4+k, p] at
        # rows (b4,p); zeros written once, the band rewritten every tile.
        s_tiles = []
        for i in range(3):
            s_i = singles.tile([TILE_B, NS, P], E3, tag=f"s{i}")
            nc.vector.memset(s_i.bitcast(F32), 0.0)
            s_tiles.append(s_i)

        for t in range(nt):
            # ---- loads ----
            x_t = xp.tile([TILE_B, NS, L], E3, tag="x_t")
            nc.sync.dma_start(out=x_t, in_=x_d[t])
            meta = small.tile([TILE_B, MW], BF, tag="meta")
            nc.gpsimd.dma_start(out=meta, in_=meta_d[t])

            # ---- fill the diagonal band of the slabs with this tile's metrics
            # element (32*b4+p, slab 4k+j, col k+8*b4): free off = 129k+32j+8b4
            s_all = s_tiles[t % 3]
            for b4 in range(4):
                view = s_all[b4 * P:(b4 + 1) * P, :, :]
                out_ap = bass.AP(
                    tensor=view.tensor, offset=view.offset + 8 * b4,
                    ap=[view.ap[0], [32, NJ], [129, NK]],
                )
                mtv = meta[b4 * P:(b4 + 1) * P, 0:P]
                in_ap = bass.AP(
                    tensor=mtv.tensor, offset=mtv.offset,
                    ap=[mtv.ap[0], [1, NJ], [4, NK]],
                )
                nc.vector.tensor_copy(out=out_ap, in_=in_ap)

            # ---- s1, s2 (f32 accumulation over bf16 metrics) ----
            mb = meta[:, P:2 * P]
            m2 = small.tile([TILE_B, P], BF, tag="m2")
            nc.vector.tensor_mul(m2, mb, mb)
            s12 = small.tile([TILE_B, 2], F32, tag="s12")  # [s1 | s2]
            nc.vector.tensor_reduce(
                out=s12[:, 0:1], in_=mb, axis=mybir.AxisListType.X,
                op=mybir.AluOpType.add,
            )
            nc.vector.tensor_reduce(
                out=s12[:, 1:2], in_=m2, axis=mybir.AxisListType.X,
                op=mybir.AluOpType.add,
            )

            # ---- z via 4 concurrent column-tiled accumulation chains ----
            psum_z = psz.tile([TILE_B, L], F32, tag="psum_z")
            for k in range(NK):
                for j in range(NJ):
                    s = 4 * k + j
                    nc.tensor.matmul(
                        out=psum_z[32 * j:32 * (j + 1), :],
                        lhsT=s_all[:, s, :],
                        rhs=x_t[:, s, :],
                        start=(k == 0),
                        stop=(k == NK - 1),
                        tile_position=(0, 32 * j),
                        skip_group_check=True,
                    )

            # ---- A = my @ WmT'  (no transpose needed: myT from host) ----
            myt0 = meta[:, 2 * P:2 * P + TILE_B]
            myt1 = meta[:, 2 * P + TILE_B:MW]
            psum_a = psa.tile([TILE_B, R], F32, tag="psum_a")
            nc.tensor.matmul(
                out=psum_a, lhsT=myt0, rhs=wm_sb[:, 0:R],
                start=True, stop=False,
            )
            nc.tensor.matmul(
                out=psum_a, lhsT=myt1, rhs=wm_sb[:, R:2 * R],
                start=False, stop=True,
            )

            # ---- transpose z (2 bf16 chunks) and s12 (f32) ----
            z_sb = small.tile([TILE_B, L], BF, tag="z_sb")
            nc.scalar.copy(out=z_sb, in_=psum_z)
            zts = []
            for c in range(2):
                ptz = pst.tile([TILE_B, TILE_B], BF, tag="ptz")
                nc.tensor.transpose(
                    out=ptz, in_=z_sb[:, c * TILE_B:(c + 1) * TILE_B],
                    identity=identb,
                )
                zt = xtp.tile([TILE_B, TILE_B], BF, tag=f"zt{c}")
                nc.scalar.copy(out=zt, in_=ptz)
                zts.append(zt)
            pts = pst.tile([TILE_B, TILE_B], F32, tag="pts", bufs=1)
            nc.tensor.transpose(out=pts[0:2, :], in_=s12, identity=identf)
            s12t = xtp.tile([2, TILE_B], BF, tag="s12t")
            nc.scalar.copy(out=s12t, in_=pts[0:2, :])

            # ---- psum_o = z@WpT' + s1*bias' + s2*wm' ----
            psum_o = pso.tile([TILE_B, R], F32, tag="psum_o")
            nc.tensor.matmul(
                out=psum_o, lhsT=zts[0], rhs=wp_sb[:, 0:R],
                start=True, stop=False,
            )
            nc.tensor.matmul(
                out=psum_o, lhsT=zts[1], rhs=wp_sb[:, R:2 * R],
                start=False, stop=False,
            )
            nc.tensor.matmul(
                out=psum_o, lhsT=s12t, rhs=w45[0:2, :],
                start=False, stop=True,
            )

            # ---- out = A_sb * s1 + psum_o  (DVE), then store ----
            a_sb = small.tile([TILE_B, R], BF, tag="a_sb")
            nc.scalar.copy(out=a_sb, in_=psum_a)
            out_sb = small.tile([TILE_B, R], F32, tag="out_sb")
            nc.vector.scalar_tensor_tensor(
                out=out_sb, in0=a_sb, scalar=s12[:, 0:1], in1=psum_o,
                op0=mybir.AluOpType.mult, op1=mybir.AluOpType.add,
            )
            nc.gpsimd.dma_start(
                out=out_d[t * TILE_B:(t + 1) * TILE_B, :], in_=out_sb
            )

    nc.compile()
    return nc


class TileCtx:
    """with TileCtx(nc) as (tc, ctx): — TileContext plus an ExitStack."""

    def __init__(self, nc):
        from contextlib import ExitStack
        self.tc = tile.TileContext(nc)
        self.ctx = ExitStack()

    def __enter__(self):
        return self.tc.__enter__(), self.ctx.__enter__()

    def __exit__(self, *a):
        self.ctx.__exit__(*a)
        return self.tc.__exit__(*a)


def prep_inputs(my_latent, peer_latents, peer_metrics, W, b, nt=NT, n_cores=N_CORES):
    """Host-side shard + layout prep (dtype casts and permutes; 1/P folded
    into the weight pack)."""
    E3np = ml_dtypes.float8_e3m4
    BFnp = ml_dtypes.bfloat16
    bc = nt * TILE_B

    wbf = np.zeros((TILE_B, 4 * L + TILE_B + R), dtype=np.float32)
    wt = np.ascontiguousarray(W.T) / P                   # [513, 256] pre-scaled
    wbf[:, 0:R] = wt[0:TILE_B]
    wbf[:, R:2 * R] = wt[TILE_B:2 * TILE_B]
    wbf[:, 2 * L:2 * L + R] = wt[L:L + TILE_B]
    wbf[:, 2 * L + R:4 * L] = wt[L + TILE_B:L + 2 * TILE_B]
    wbf[:, 4 * L:4 * L + TILE_B] = np.eye(TILE_B, dtype=np.float32)
    wbf[0, 4 * L + TILE_B:] = b / P                      # bias'
    wbf[1, 4 * L + TILE_B:] = wt[2 * L]                  # wm'
    wbf = wbf.astype(BFnp)
    identf = np.eye(TILE_B, dtype=np.float32)

    # batch scramble within a tile: b_loc = 32j + 8*b4 + k
    b4r = np.arange(4)[:, None, None]
    kr = np.arange(NK)[None, :, None]
    jr = np.arange(NJ)[None, None, :]
    bl_map = 32 * jr + 8 * b4r + kr                      # [4, 8, 4] (b4, k, j)

    in_maps = []
    for c in range(n_cores):
        sl = slice(c * bc, (c + 1) * bc)
        peer_c = peer_latents[sl].reshape(nt, TILE_B, P, L)
        m_c = peer_metrics[sl].reshape(nt, TILE_B, P)
        my_c = my_latent[sl].reshape(nt, TILE_B, L)

        # x[t, 32*b4+p, 4k+j, l] = peer[bl_map, p, l]
        xc = peer_c[:, bl_map, :, :]                     # [nt, 4, 8, 4, P, L]
        xc = np.ascontiguousarray(
            xc.transpose(0, 1, 4, 2, 3, 5)               # t, b4, p, k, j, l
        ).reshape(nt, TILE_B, NS, L).astype(E3np)

        # meta: [m_t | mb | myT]; m_t[t, 32*b4+p, 4k+j] = m[bl_map, p]
        meta = np.empty((nt, TILE_B, MW), dtype=np.float32)
        mt = m_c[:, bl_map, :]                           # [nt, 4, 8, 4, P]
        meta[:, :, 0:P] = mt.transpose(0, 1, 4, 2, 3).reshape(nt, TILE_B, P)
        meta[:, :, P:2 * P] = m_c
        # myT[t, lp, c2*128 + b] = my[b, c2*128+lp]
        meta[:, :, 2 * P:] = my_c.reshape(nt, TILE_B, 2, TILE_B).transpose(
            0, 3, 2, 1).reshape(nt, TILE_B, 2 * TILE_B)
        meta = meta.astype(BFnp)

        in_maps.append({
            "x": xc,
            "meta": meta,
            "wbf": wbf,
            "identf": identf,
        })
    return in_maps


def run(my_latent, peer_latents, peer_metrics, W, b, trace=False, **kw):
    if "nc" not in _cache:
        _cache["nc"] = build_bass()
    nc = _cache["nc"]
    in_maps = prep_inputs(
        np.asarray(my_latent, dtype=np.float32),
        np.asarray(peer_latents, dtype=np.float32),
        np.asarray(peer_metrics, dtype=np.float32),
        np.asarray(W, dtype=np.float32),
        np.asarray(b, dtype=np.float32),
    )
    res = run_bass_kernel_spmd(
        nc, in_maps, core_ids=list(range(N_CORES)), trace=trace, **kw
    )
    out = np.concatenate([r["out"] for r in res.results], axis=0)
    return out, res


def kernel(my_latent, peer_latents, peer_metrics, W, b):
    out, _ = run(my_latent, peer_latents, peer_metrics, W, b)
    return out


# revision 9
# speedup vs baseline: 1.8800x; 1.0189x over previous
"""Trainium2 Bass kernel for nn_MiddleOut (gnn_message_passing).

Math (reference), with P = #peers and W = [Wm | Wp | wm] along the in dim:
    out = (1/P) * [ s1*(my@Wm.T + bias) + z@Wp.T + s2*wm ]
    s1[b] = sum_p m[b,p];  s2[b] = sum_p m[b,p]^2
    z[b,l] = sum_p m[b,p] * peer[b,p,l]

Sharding: pure data parallel over batch across 8 cores (2048 rows/core,
16 tiles of 128).

On-device strategy per tile:
  - peers are cast to fp8 e3m4 on host (4 mantissa bits; |peer|max ~5.4 well
    under the 15.5 format max), halving the dominant HBM stream vs bf16;
    measured end-to-end absmax-rel ~4.3e-3 vs the 2e-2 budget.
  - the weighted peer-reduction z runs as FOUR CONCURRENT column-tiled
    matmul chains (tile_position col groups j=0..3, 32 output partitions
    each): stationary = a [128,32] metric band slab, moving = the [128,256]
    peer slab. Column tiling streams 4 moving operands through the PE at
    once, cutting the z wall-time ~4x vs one 128-wide chain.
    Batch mapping: b_loc = 32*j + 8*b4 + k, slab s = 4*k + j, band column
    c = k + 8*b4 holds m[b_loc, p] at rows (b4,p).
  - projection avoids transposing [u|z]: my is host-transposed (myT) so
    A = my@Wm'/P comes from 2 direct chains; only z (2 chunks, bf16) and
    s12 (tiny) are PE-transposed, then 3 chains give
    psum_o = z@Wp' + s1*bias' + s2*wm' (all 1/P pre-folded on host).
  - final combine on DVE: out = A_sb * s1 + psum_o via scalar_tensor_tensor
    (s1, s2 are f32 DVE reductions of the bf16 metrics).
  - DMA issue cost (~0.6us of sequencer time per dma_start) is kept off the
    busy engines: x is one 1MB issue on sync; the merged meta|myT load and
    the out store issue from the otherwise-idle GpSimd (SWDGE).
"""

import ml_dtypes
import numpy as np

import concourse.bass as bass
import concourse.mybir as mybir
import concourse.tile as tile
from concourse import bacc
from concourse.bass_utils import run_bass_kernel_spmd

F32 = mybir.dt.float32
BF = mybir.dt.bfloat16
E3 = mybir.dt.float8e3

B, P, L, R = 16384, 32, 256, 256
N_CORES = 8
BC = B // N_CORES          # 2048 batches per core
TILE_B = 128               # batches per SBUF tile
NT = BC // TILE_B          # 16 tiles
NJ = 4                     # column-tile groups (32 out partitions each)
NK = 8                     # chained matmuls per column group
NS = NJ * NK               # 32 peer slabs per tile
MW = 2 * P + 2 * TILE_B    # meta row: [m_t | mb | myT chunk0 | myT chunk1]

_cache = {}


def build_bass(nt=NT, num_devices=N_CORES):
    bc = nt * TILE_B
    nc = bacc.Bacc(
        "TRN2", target_bir_lowering=False, debug=False, num_devices=num_devices
    )

    x_d = nc.dram_tensor("x", [nt, TILE_B, NS, L], E3, kind="ExternalInput")
    meta_d = nc.dram_tensor("meta", [nt, TILE_B, MW], BF, kind="ExternalInput")
    # wbf packs [WmT' | WpT' | identity | w45'] (bf16); identf is f32 identity
    wbf_d = nc.dram_tensor("wbf", [TILE_B, 4 * L + TILE_B + R], BF, kind="ExternalInput")
    identf_d = nc.dram_tensor("identf", [TILE_B, TILE_B], F32, kind="ExternalInput")
    out_d = nc.dram_tensor("out", [bc, R], F32, kind="ExternalOutput")

    with TileCtx(nc) as (tc, ctx):
        singles = ctx.enter_context(tc.tile_pool(name="singles", bufs=1))
        xp = ctx.enter_context(tc.tile_pool(name="xp", bufs=5))
        small = ctx.enter_context(tc.tile_pool(name="small", bufs=4))
        xtp = ctx.enter_context(tc.tile_pool(name="xtp", bufs=3))
        psz = ctx.enter_context(tc.tile_pool(name="psz", bufs=2, space="PSUM"))
        pst = ctx.enter_context(tc.tile_pool(name="pst", bufs=2, space="PSUM"))
        pso = ctx.enter_context(tc.tile_pool(name="pso", bufs=2, space="PSUM"))
        psa = ctx.enter_context(tc.tile_pool(name="psa", bufs=1, space="PSUM"))

        w_sb = singles.tile([TILE_B, 4 * L + TILE_B + R], BF)
        nc.gpsimd.dma_start(out=w_sb, in_=wbf_d[:, :])
        identf = singles.tile([TILE_B, TILE_B], F32)
        nc.gpsimd.dma_start(out=identf, in_=identf_d[:, :])
        wm_sb = w_sb[:, 0:2 * L]                     # [128, 2*256] WmT'/P
        wp_sb = w_sb[:, 2 * L:4 * L]                 # [128, 2*256] WpT'/P
        identb = w_sb[:, 4 * L:4 * L + TILE_B]       # [128, 128] bf16 identity
        w45 = w_sb[:, 4 * L + TILE_B:]               # rows 0:2 = [bias'; wm']/P

        # Ping-pong block-diagonal stationaries for the weighted peer-reduce.
        # Slab s = 4k+j is [128, 32]: column k+8*b4 holds m[32j+8b4+k, p] at
        # rows (b4,p); zeros written once, the band rewritten every tile.
        s_tiles = []
        for i in range(3):
            s_i = singles.tile([TILE_B, NS, P], E3, tag=f"s{i}")
            nc.vector.memset(s_i.bitcast(F32), 0.0)
            s_tiles.append(s_i)

        # Two-stage software pipeline over the PE stream so its queue never
        # head-of-line blocks on ACT evacuations: per iteration i the PE gets
        # [chains(i-2), z(i), A(i), transposes(i-1)] — every group is ready
        # when it reaches the queue head, keeping the PE dense (HAM warm).
        # The metric band for tile i+1 is DVE-written one stage early so the
        # next iteration's z never waits on it.
        st = {}  # per-tile live tiles

        def load_meta(t):
            meta = small.tile([TILE_B, MW], BF, tag="meta")
            nc.gpsimd.dma_start(out=meta, in_=meta_d[t])
            st[t] = {"meta": meta}

        def band_s12(t):
            meta = st[t]["meta"]
            # band: element (32*b4+p, slab 4k+j, col k+8*b4): off = 129k+32j+8b4
            s_all = s_tiles[t % 3]
            for b4 in range(4):
                view = s_all[b4 * P:(b4 + 1) * P, :, :]
                out_ap = bass.AP(
                    tensor=view.tensor, offset=view.offset + 8 * b4,
                    ap=[view.ap[0], [32, NJ], [129, NK]],
                )
                mtv = meta[b4 * P:(b4 + 1) * P, 0:P]
                in_ap = bass.AP(
                    tensor=mtv.tensor, offset=mtv.offset,
                    ap=[mtv.ap[0], [1, NJ], [4, NK]],
                )
                nc.vector.tensor_copy(out=out_ap, in_=in_ap)
            mb = meta[:, P:2 * P]
            m2 = small.tile([TILE_B, P], BF, tag="m2")
            nc.vector.tensor_mul(m2, mb, mb)
            s12 = small.tile([TILE_B, 2], F32, tag="s12")  # [s1 | s2]
            nc.vector.tensor_reduce(
                out=s12[:, 0:1], in_=mb, axis=mybir.AxisListType.X,
                op=mybir.AluOpType.add,
            )
            nc.vector.tensor_reduce(
                out=s12[:, 1:2], in_=m2, axis=mybir.AxisListType.X,
                op=mybir.AluOpType.add,
            )
            st[t]["s12"] = s12

        def z_and_a(t):
            x_t = xp.tile([TILE_B, NS, L], E3, tag="x_t")
            nc.sync.dma_start(out=x_t, in_=x_d[t])
            s_all = s_tiles[t % 3]
            psum_z = psz.tile([TILE_B, L], F32, tag="psum_z")
            for k in range(NK):
                for j in range(NJ):
                    s = 4 * k + j
                    nc.tensor.matmul(
                        out=psum_z[32 * j:32 * (j + 1), :],
                        lhsT=s_all[:, s, :],
                        rhs=x_t[:, s, :],
                        start=(k == 0),
                        stop=(k == NK - 1),
                        tile_position=(0, 32 * j),
                        skip_group_check=True,
                    )
            meta = st[t]["meta"]
            psum_a = psa.tile([TILE_B, R], F32, tag="psum_a")
            nc.tensor.matmul(
                out=psum_a, lhsT=meta[:, 2 * P:2 * P + TILE_B],
                rhs=wm_sb[:, 0:R], start=True, stop=False,
            )
            nc.tensor.matmul(
                out=psum_a, lhsT=meta[:, 2 * P + TILE_B:MW],
                rhs=wm_sb[:, R:2 * R], start=False, stop=True,
            )
            z_sb = small.tile([TILE_B, L], BF, tag="z_sb")
            nc.scalar.copy(out=z_sb, in_=psum_z)
            a_sb = small.tile([TILE_B, R], BF, tag="a_sb")
            nc.scalar.copy(out=a_sb, in_=psum_a)
            st[t]["z_sb"] = z_sb
            st[t]["a_sb"] = a_sb

        def transposes(t):
            z_sb, s12 = st[t]["z_sb"], st[t]["s12"]
            zts = []
            for c in range(2):
                ptz = pst.tile([TILE_B, TILE_B], BF, tag="ptz")
                nc.tensor.transpose(
                    out=ptz, in_=z_sb[:, c * TILE_B:(c + 1) * TILE_B],
                    identity=identb,
                )
                zt = xtp.tile([TILE_B, TILE_B], BF, tag=f"zt{c}")
                nc.scalar.copy(out=zt, in_=ptz)
                zts.append(zt)
            pts = pst.tile([TILE_B, TILE_B], F32, tag="pts", bufs=1)
            nc.tensor.transpose(out=pts[0:2, :], in_=s12, identity=identf)
            s12t = xtp.tile([2, TILE_B], BF, tag="s12t")
            nc.scalar.copy(out=s12t, in_=pts[0:2, :])
            st[t]["zts"] = zts
            st[t]["s12t"] = s12t

        def chains_out(t):
            zts, s12t, s12, a_sb = (
                st[t]["zts"], st[t]["s12t"], st[t]["s12"], st[t]["a_sb"]
            )
            psum_o = pso.tile([TILE_B, R], F32, tag="psum_o")
            nc.tensor.matmul(
                out=psum_o, lhsT=zts[0], rhs=wp_sb[:, 0:R],
                start=True, stop=False,
            )
            nc.tensor.matmul(
                out=psum_o, lhsT=zts[1], rhs=wp_sb[:, R:2 * R],
                start=False, stop=False,
            )
            nc.tensor.matmul(
                out=psum_o, lhsT=s12t, rhs=w45[0:2, :],
                start=False, stop=True,
            )
            out_sb = small.tile([TILE_B, R], F32, tag="out_sb")
            nc.vector.scalar_tensor_tensor(
                out=out_sb, in0=a_sb, scalar=s12[:, 0:1], in1=psum_o,
                op0=mybir.AluOpType.mult, op1=mybir.AluOpType.add,
            )
            nc.gpsimd.dma_start(
                out=out_d[t * TILE_B:(t + 1) * TILE_B, :], in_=out_sb
            )
            del st[t]

        load_meta(0)
        band_s12(0)
        for i in range(nt + 2):
            if 2 <= i:
                chains_out(i - 2)
            if i < nt:
                if i + 1 < nt:
                    load_meta(i + 1)
                z_and_a(i)
                if i + 1 < nt:
                    band_s12(i + 1)
            if 1 <= i <= nt:
                transposes(i - 1)

    nc.compile()
    return nc


class TileCtx:
    """with TileCtx(nc) as (tc, ctx): — TileContext plus an ExitStack."""

    def __init__(self, nc):
        from contextlib import ExitStack
        self.tc = tile.TileContext(nc)
        self.ctx = ExitStack()

    def __enter__(self):
        return self.tc.__enter__(), self.ctx.__enter__()

    def __exit__(self, *a):
        self.ctx.__exit__(*a)
        return self.tc.__exit__(*a)


def prep_inputs(my_latent, peer_latents, peer_metrics, W, b, nt=NT, n_cores=N_CORES):
    """Host-side shard + layout prep (dtype casts and permutes; 1/P folded
    into the weight pack)."""
    E3np = ml_dtypes.float8_e3m4
    BFnp = ml_dtypes.bfloat16
    bc = nt * TILE_B

    wbf = np.zeros((TILE_B, 4 * L + TILE_B + R), dtype=np.float32)
    wt = np.ascontiguousarray(W.T) / P                   # [513, 256] pre-scaled
    wbf[:, 0:R] = wt[0:TILE_B]
    wbf[:, R:2 * R] = wt[TILE_B:2 * TILE_B]
    wbf[:, 2 * L:2 * L + R] = wt[L:L + TILE_B]
    wbf[:, 2 * L + R:4 * L] = wt[L + TILE_B:L + 2 * TILE_B]
    wbf[:, 4 * L:4 * L + TILE_B] = np.eye(TILE_B, dtype=np.float32)
    wbf[0, 4 * L + TILE_B:] = b / P                      # bias'
    wbf[1, 4 * L + TILE_B:] = wt[2 * L]                  # wm'
    wbf = wbf.astype(BFnp)
    identf = np.eye(TILE_B, dtype=np.float32)

    # batch scramble within a tile: b_loc = 32j + 8*b4 + k
    b4r = np.arange(4)[:, None, None]
    kr = np.arange(NK)[None, :, None]
    jr = np.arange(NJ)[None, None, :]
    bl_map = 32 * jr + 8 * b4r + kr                      # [4, 8, 4] (b4, k, j)

    in_maps = []
    for c in range(n_cores):
        sl = slice(c * bc, (c + 1) * bc)
        peer_c = peer_latents[sl].reshape(nt, TILE_B, P, L)
        m_c = peer_metrics[sl].reshape(nt, TILE_B, P)
        my_c = my_latent[sl].reshape(nt, TILE_B, L)

        # x[t, 32*b4+p, 4k+j, l] = peer[bl_map, p, l]
        xc = peer_c[:, bl_map, :, :]                     # [nt, 4, 8, 4, P, L]
        xc = np.ascontiguousarray(
            xc.transpose(0, 1, 4, 2, 3, 5)               # t, b4, p, k, j, l
        ).reshape(nt, TILE_B, NS, L).astype(E3np)

        # meta: [m_t | mb | myT]; m_t[t, 32*b4+p, 4k+j] = m[bl_map, p]
        meta = np.empty((nt, TILE_B, MW), dtype=np.float32)
        mt = m_c[:, bl_map, :]                           # [nt, 4, 8, 4, P]
        meta[:, :, 0:P] = mt.transpose(0, 1, 4, 2, 3).reshape(nt, TILE_B, P)
        meta[:, :, P:2 * P] = m_c
        # myT[t, lp, c2*128 + b] = my[b, c2*128+lp]
        meta[:, :, 2 * P:] = my_c.reshape(nt, TILE_B, 2, TILE_B).transpose(
            0, 3, 2, 1).reshape(nt, TILE_B, 2 * TILE_B)
        meta = meta.astype(BFnp)

        in_maps.append({
            "x": xc,
            "meta": meta,
            "wbf": wbf,
            "identf": identf,
        })
    return in_maps


def run(my_latent, peer_latents, peer_metrics, W, b, trace=False, **kw):
    if "nc" not in _cache:
        _cache["nc"] = build_bass()
    nc = _cache["nc"]
    in_maps = prep_inputs(
        np.asarray(my_latent, dtype=np.float32),
        np.asarray(peer_latents, dtype=np.float32),
        np.asarray(peer_metrics, dtype=np.float32),
        np.asarray(W, dtype=np.float32),
        np.asarray(b, dtype=np.float32),
    )
    res = run_bass_kernel_spmd(
        nc, in_maps, core_ids=list(range(N_CORES)), trace=trace, **kw
    )
    out = np.concatenate([r["out"] for r in res.results], axis=0)
    return out, res


def kernel(my_latent, peer_latents, peer_metrics, W, b):
    out, _ = run(my_latent, peer_latents, peer_metrics, W, b)
    return out


# revision 11
# speedup vs baseline: 2.0996x; 1.1168x over previous
"""Trainium2 Bass kernel for nn_MiddleOut (gnn_message_passing).

Math (reference), with P = #peers and W = [Wm | Wp | wm] along the in dim:
    out = (1/P) * [ s1*(my@Wm.T + bias) + z@Wp.T + s2*wm ]
    s1[b] = sum_p m[b,p];  s2[b] = sum_p m[b,p]^2
    z[b,l] = sum_p m[b,p] * peer[b,p,l]

Sharding: pure data parallel over batch across 8 cores (2048 rows/core,
16 tiles of 128).

On-device strategy per tile:
  - peers are cast to fp8 e3m4 on host (4 mantissa bits; |peer|max ~5.4 well
    under the 15.5 format max), halving the dominant HBM stream vs bf16;
    measured end-to-end absmax-rel ~4.3e-3 vs the 2e-2 budget.
  - the weighted peer-reduction z runs as FOUR CONCURRENT column-tiled
    matmul chains (tile_position col groups j=0..3, 32 output partitions
    each): stationary = a [128,32] metric band slab, moving = the [128,256]
    peer slab. Column tiling streams 4 moving operands through the PE at
    once, cutting the z wall-time ~4x vs one 128-wide chain.
    Batch mapping: b_loc = 32*j + 8*b4 + k, slab s = 4*k + j, band column
    c = k + 8*b4 holds m[b_loc, p] at rows (b4,p).
  - projection avoids transposing [u|z]: my is host-transposed (myT) so
    A = my@Wm'/P comes from 2 direct chains; only z (2 chunks, bf16) and
    s12 (tiny) are PE-transposed, then 3 chains give
    psum_o = z@Wp' + s1*bias' + s2*wm' (all 1/P pre-folded on host).
  - final combine on DVE: out = A_sb * s1 + psum_o via scalar_tensor_tensor
    (s1, s2 are f32 DVE reductions of the bf16 metrics).
  - DMA issue cost (~0.6us of sequencer time per dma_start) is kept off the
    busy engines: x is one 1MB issue on sync; the merged meta|myT load and
    the out store issue from the otherwise-idle GpSimd (SWDGE).
"""

import ml_dtypes
import numpy as np

import concourse.bass as bass
import concourse.mybir as mybir
import concourse.tile as tile
from concourse import bacc
from concourse.bass_utils import run_bass_kernel_spmd

F32 = mybir.dt.float32
BF = mybir.dt.bfloat16
E3 = mybir.dt.float8e3

B, P, L, R = 16384, 32, 256, 256
N_CORES = 8
BC = B // N_CORES          # 2048 batches per core
TILE_B = 128               # batches per SBUF tile
NT = BC // TILE_B          # 16 tiles
NJ = 4                     # column-tile groups (32 out partitions each)
NK = 8                     # chained matmuls per column group
NS = NJ * NK               # 32 peer slabs per tile
MW = 2 * P + 2 * TILE_B    # meta row: [m_t | mb | myT chunk0 | myT chunk1]

_cache = {}


def build_bass(nt=NT, num_devices=N_CORES):
    bc = nt * TILE_B
    nc = bacc.Bacc(
        "TRN2", target_bir_lowering=False, debug=False, num_devices=num_devices
    )

    x_d = nc.dram_tensor("x", [nt, TILE_B, NS, L], E3, kind="ExternalInput")
    meta_d = nc.dram_tensor("meta", [nt, TILE_B, MW], BF, kind="ExternalInput")
    # wbf packs [WmT' | WpT' | identity | w45'] (bf16); identf is f32 identity
    wbf_d = nc.dram_tensor("wbf", [TILE_B, 4 * L + TILE_B + R], BF, kind="ExternalInput")
    identf_d = nc.dram_tensor("identf", [TILE_B, TILE_B], F32, kind="ExternalInput")
    out_d = nc.dram_tensor("out", [bc, R], BF, kind="ExternalOutput")

    with TileCtx(nc) as (tc, ctx):
        singles = ctx.enter_context(tc.tile_pool(name="singles", bufs=1))
        xp = ctx.enter_context(tc.tile_pool(name="xp", bufs=6))
        small = ctx.enter_context(tc.tile_pool(name="small", bufs=4))
        xtp = ctx.enter_context(tc.tile_pool(name="xtp", bufs=3))
        psz = ctx.enter_context(tc.tile_pool(name="psz", bufs=2, space="PSUM"))
        pst = ctx.enter_context(tc.tile_pool(name="pst", bufs=2, space="PSUM"))
        pso = ctx.enter_context(tc.tile_pool(name="pso", bufs=2, space="PSUM"))
        psa = ctx.enter_context(tc.tile_pool(name="psa", bufs=1, space="PSUM"))

        w_sb = singles.tile([TILE_B, 4 * L + TILE_B + R], BF)
        identf = singles.tile([TILE_B, TILE_B], F32)
        wm_sb = w_sb[:, 0:2 * L]                     # [128, 2*256] WmT'/P
        wp_sb = w_sb[:, 2 * L:4 * L]                 # [128, 2*256] WpT'/P
        identb = w_sb[:, 4 * L:4 * L + TILE_B]       # [128, 128] bf16 identity
        w45 = w_sb[:, 4 * L + TILE_B:]               # rows 0:2 = [bias'; wm']/P

        # Ping-pong block-diagonal stationaries for the weighted peer-reduce.
        # Slab s = 4k+j is [128, 32]: column k+8*b4 holds m[32j+8b4+k, p] at
        # rows (b4,p); zeros written once, the band rewritten every tile.
        s_tiles = []
        for i in range(3):
            s_i = singles.tile([TILE_B, NS, P], E3, tag=f"s{i}")
            nc.vector.memset(s_i.bitcast(F32), 0.0)
            s_tiles.append(s_i)

        # Two-stage software pipeline over the PE stream so its queue never
        # head-of-line blocks on ACT evacuations: per iteration i the PE gets
        # [chains(i-2), z(i), A(i), transposes(i-1)] — every group is ready
        # when it reaches the queue head, keeping the PE dense (HAM warm).
        # The metric band for tile i+1 is DVE-written one stage early so the
        # next iteration's z never waits on it.
        st = {}  # per-tile live tiles

        def load_meta(t):
            meta = small.tile([TILE_B, MW], BF, tag="meta")
            nc.sync.dma_start(out=meta, in_=meta_d[t])
            st[t] = {"meta": meta}

        def band_s12(t):
            meta = st[t]["meta"]
            # band: element (32*b4+p, slab 4k+j, col k+8*b4): off = 129k+32j+8b4
            s_all = s_tiles[t % 3]
            for b4 in range(4):
                view = s_all[b4 * P:(b4 + 1) * P, :, :]
                out_ap = bass.AP(
                    tensor=view.tensor, offset=view.offset + 8 * b4,
                    ap=[view.ap[0], [32, NJ], [129, NK]],
                )
                mtv = meta[b4 * P:(b4 + 1) * P, 0:P]
                in_ap = bass.AP(
                    tensor=mtv.tensor, offset=mtv.offset,
                    ap=[mtv.ap[0], [1, NJ], [4, NK]],
                )
                nc.vector.tensor_copy(out=out_ap, in_=in_ap)
            mb = meta[:, P:2 * P]
            m2 = small.tile([TILE_B, P], BF, tag="m2")
            nc.vector.tensor_mul(m2, mb, mb)
            s12 = small.tile([TILE_B, 2], F32, tag="s12")  # [s1 | s2]
            nc.vector.tensor_reduce(
                out=s12[:, 0:1], in_=mb, axis=mybir.AxisListType.X,
                op=mybir.AluOpType.add,
            )
            nc.vector.tensor_reduce(
                out=s12[:, 1:2], in_=m2, axis=mybir.AxisListType.X,
                op=mybir.AluOpType.add,
            )
            st[t]["s12"] = s12

        def z_and_a(t):
            x_t = xp.tile([TILE_B, NS, L], E3, tag="x_t")
            # chunk the first loads so tile-0/1 matmuls start on partial data
            nch = 4 if t == 0 else (2 if t == 1 else 1)
            step = NS // nch
            for q in range(nch):
                nc.sync.dma_start(
                    out=x_t[:, q * step:(q + 1) * step, :],
                    in_=x_d[t, :, q * step:(q + 1) * step, :],
                )
            s_all = s_tiles[t % 3]
            psum_z = psz.tile([TILE_B, L], F32, tag="psum_z")
            for k in range(NK):
                for j in range(NJ):
                    s = 4 * k + j
                    nc.tensor.matmul(
                        out=psum_z[32 * j:32 * (j + 1), :],
                        lhsT=s_all[:, s, :],
                        rhs=x_t[:, s, :],
                        start=(k == 0),
                        stop=(k == NK - 1),
                        tile_position=(0, 32 * j),
                        skip_group_check=True,
                    )
            meta = st[t]["meta"]
            psum_a = psa.tile([TILE_B, R], F32, tag="psum_a")
            nc.tensor.matmul(
                out=psum_a, lhsT=meta[:, 2 * P:2 * P + TILE_B],
                rhs=wm_sb[:, 0:R], start=True, stop=False,
            )
            nc.tensor.matmul(
                out=psum_a, lhsT=meta[:, 2 * P + TILE_B:MW],
                rhs=wm_sb[:, R:2 * R], start=False, stop=True,
            )
            z_sb = small.tile([TILE_B, L], BF, tag="z_sb")
            nc.scalar.copy(out=z_sb, in_=psum_z)
            a_sb = small.tile([TILE_B, R], BF, tag="a_sb")
            nc.scalar.copy(out=a_sb, in_=psum_a)
            st[t]["z_sb"] = z_sb
            st[t]["a_sb"] = a_sb

        def transposes(t):
            z_sb, s12 = st[t]["z_sb"], st[t]["s12"]
            zts = []
            for c in range(2):
                ptz = pst.tile([TILE_B, TILE_B], BF, tag="ptz")
                nc.tensor.transpose(
                    out=ptz, in_=z_sb[:, c * TILE_B:(c + 1) * TILE_B],
                    identity=identb,
                )
                zt = xtp.tile([TILE_B, TILE_B], BF, tag=f"zt{c}")
                nc.scalar.copy(out=zt, in_=ptz)
                zts.append(zt)
            pts = pst.tile([TILE_B, TILE_B], F32, tag="pts", bufs=1)
            nc.tensor.transpose(out=pts[0:2, :], in_=s12, identity=identf)
            s12t = xtp.tile([2, TILE_B], BF, tag="s12t")
            nc.scalar.copy(out=s12t, in_=pts[0:2, :])
            st[t]["zts"] = zts
            st[t]["s12t"] = s12t

        def chains_out(t):
            zts, s12t, s12, a_sb = (
                st[t]["zts"], st[t]["s12t"], st[t]["s12"], st[t]["a_sb"]
            )
            psum_o = pso.tile([TILE_B, R], F32, tag="psum_o")
            nc.tensor.matmul(
                out=psum_o, lhsT=zts[0], rhs=wp_sb[:, 0:R],
                start=True, stop=False,
            )
            nc.tensor.matmul(
                out=psum_o, lhsT=zts[1], rhs=wp_sb[:, R:2 * R],
                start=False, stop=False,
            )
            nc.tensor.matmul(
                out=psum_o, lhsT=s12t, rhs=w45[0:2, :],
                start=False, stop=True,
            )
            out_sb = small.tile([TILE_B, R], BF, tag="out_sb")
            nc.vector.scalar_tensor_tensor(
                out=out_sb, in0=a_sb, scalar=s12[:, 0:1], in1=psum_o,
                op0=mybir.AluOpType.mult, op1=mybir.AluOpType.add,
            )
            nc.gpsimd.dma_start(
                out=out_d[t * TILE_B:(t + 1) * TILE_B, :], in_=out_sb
            )
            del st[t]

        load_meta(0)
        nc.sync.dma_start(out=w_sb, in_=wbf_d[:, :])
        nc.sync.dma_start(out=identf, in_=identf_d[:, :])
        band_s12(0)
        for i in range(nt + 2):
            if 2 <= i:
                chains_out(i - 2)
            if i < nt:
                if i + 1 < nt:
                    load_meta(i + 1)
                z_and_a(i)
                if i + 1 < nt:
                    band_s12(i + 1)
            if 1 <= i <= nt:
                transposes(i - 1)

    nc.compile()
    return nc


class TileCtx:
    """with TileCtx(nc) as (tc, ctx): — TileContext plus an ExitStack."""

    def __init__(self, nc):
        from contextlib import ExitStack
        self.tc = tile.TileContext(nc)
        self.ctx = ExitStack()

    def __enter__(self):
        return self.tc.__enter__(), self.ctx.__enter__()

    def __exit__(self, *a):
        self.ctx.__exit__(*a)
        return self.tc.__exit__(*a)


def prep_inputs(my_latent, peer_latents, peer_metrics, W, b, nt=NT, n_cores=N_CORES):
    """Host-side shard + layout prep (dtype casts and permutes; 1/P folded
    into the weight pack)."""
    E3np = ml_dtypes.float8_e3m4
    BFnp = ml_dtypes.bfloat16
    bc = nt * TILE_B

    wbf = np.zeros((TILE_B, 4 * L + TILE_B + R), dtype=np.float32)
    wt = np.ascontiguousarray(W.T) / P                   # [513, 256] pre-scaled
    wbf[:, 0:R] = wt[0:TILE_B]
    wbf[:, R:2 * R] = wt[TILE_B:2 * TILE_B]
    wbf[:, 2 * L:2 * L + R] = wt[L:L + TILE_B]
    wbf[:, 2 * L + R:4 * L] = wt[L + TILE_B:L + 2 * TILE_B]
    wbf[:, 4 * L:4 * L + TILE_B] = np.eye(TILE_B, dtype=np.float32)
    wbf[0, 4 * L + TILE_B:] = b / P                      # bias'
    wbf[1, 4 * L + TILE_B:] = wt[2 * L]                  # wm'
    wbf = wbf.astype(BFnp)
    identf = np.eye(TILE_B, dtype=np.float32)

    # batch scramble within a tile: b_loc = 32j + 8*b4 + k
    b4r = np.arange(4)[:, None, None]
    kr = np.arange(NK)[None, :, None]
    jr = np.arange(NJ)[None, None, :]
    bl_map = 32 * jr + 8 * b4r + kr                      # [4, 8, 4] (b4, k, j)

    in_maps = []
    for c in range(n_cores):
        sl = slice(c * bc, (c + 1) * bc)
        peer_c = peer_latents[sl].reshape(nt, TILE_B, P, L)
        m_c = peer_metrics[sl].reshape(nt, TILE_B, P)
        my_c = my_latent[sl].reshape(nt, TILE_B, L)

        # x[t, 32*b4+p, 4k+j, l] = peer[bl_map, p, l]
        xc = peer_c[:, bl_map, :, :]                     # [nt, 4, 8, 4, P, L]
        xc = np.ascontiguousarray(
            xc.transpose(0, 1, 4, 2, 3, 5)               # t, b4, p, k, j, l
        ).reshape(nt, TILE_B, NS, L).astype(E3np)

        # meta: [m_t | mb | myT]; m_t[t, 32*b4+p, 4k+j] = m[bl_map, p]
        meta = np.empty((nt, TILE_B, MW), dtype=np.float32)
        mt = m_c[:, bl_map, :]                           # [nt, 4, 8, 4, P]
        meta[:, :, 0:P] = mt.transpose(0, 1, 4, 2, 3).reshape(nt, TILE_B, P)
        meta[:, :, P:2 * P] = m_c
        # myT[t, lp, c2*128 + b] = my[b, c2*128+lp]
        meta[:, :, 2 * P:] = my_c.reshape(nt, TILE_B, 2, TILE_B).transpose(
            0, 3, 2, 1).reshape(nt, TILE_B, 2 * TILE_B)
        meta = meta.astype(BFnp)

        in_maps.append({
            "x": xc,
            "meta": meta,
            "wbf": wbf,
            "identf": identf,
        })
    return in_maps


def run(my_latent, peer_latents, peer_metrics, W, b, trace=False, **kw):
    if "nc" not in _cache:
        _cache["nc"] = build_bass()
    nc = _cache["nc"]
    in_maps = prep_inputs(
        np.asarray(my_latent, dtype=np.float32),
        np.asarray(peer_latents, dtype=np.float32),
        np.asarray(peer_metrics, dtype=np.float32),
        np.asarray(W, dtype=np.float32),
        np.asarray(b, dtype=np.float32),
    )
    res = run_bass_kernel_spmd(
        nc, in_maps, core_ids=list(range(N_CORES)), trace=trace, **kw
    )
    out = np.concatenate([r["out"] for r in res.results], axis=0).astype(np.float32)
    return out, res


def kernel(my_latent, peer_latents, peer_metrics, W, b):
    out, _ = run(my_latent, peer_latents, peer_metrics, W, b)
    return out
